# revision 1
# baseline (speedup 1.0000x reference)
"""Cost-adjusted cross-entropy loss on 8 Trainium2 NeuronCores (Bass/Tile).

Math (per sample i of N=65536, C=1000 classes):
    t_i   = super_classes[target_i]
    p_i   = argmax_c logits[i, c]
    w_i   = class_weights[t_i, p_i]
    lse_i = log(sum_c exp(logits[i, c]))        # logits ~ N(0,1): no max-shift needed
    l2_i  = w_i * (lse_i - logits[i, t_i])      # == -w_i * log_softmax(logits)[i, t_i]
    out   = sum(l2) / sum(w)

Sharding: data-parallel over N. Each core gets 8192 rows of logits/target;
class_weights and super_classes are replicated. Per-core kernel emits [128, 2]
per-partition partial (numerator, denominator) sums; the host adds the 8x128
partials and divides.

Per-core device plan:
  - target/super_classes arrive as int32 pairs (host reinterprets the int64
    bytes; values < 1000 so the low words carry everything).
  - t_i, logits[i, t_i] and class_weights[t_i, p_i] are fetched with
    indirect-DMA element gathers (8192 descriptors each) on the Pool engine.
  - The streaming loop (64 tiles of [128 rows, 1000]) uses:
      HWDGE DMA   : tile load (512 KB)
      ScalarE     : exp with row-sum accumulation -> sumexp
      VectorE     : max8 + max_index -> argmax index
  - Tail: ln(sumexp), w*(lse-g) reductions, DMA out.
"""

import numpy as np

import concourse.bass as bass
import concourse.bacc as bacc
import concourse.tile as tile
from concourse import mybir
from concourse.bass_utils import run_bass_kernel_spmd

N, C = 65536, 1000
NCORES = 8
R = N // NCORES          # rows per core
P = 128                  # partitions
TILES = R // P           # row-tiles per core

F32 = mybir.dt.float32
I32 = mybir.dt.int32
U32 = mybir.dt.uint32
AX = mybir.AxisListType.X
ALU = mybir.AluOpType
AF = mybir.ActivationFunctionType


def build_kernel(tc, x, t32, sc32, cw, out):
    nc = tc.nc
    with (  # noqa
        tc.tile_pool(name="singles", bufs=1) as singles,
        tc.tile_pool(name="xp", bufs=4) as xp,
        tc.tile_pool(name="ep", bufs=2, space="PSUM") as ep,
        tc.tile_pool(name="v8p", bufs=4) as v8p,
    ):
        # Persistent per-sample accumulators, laid out [partition, tile].
        sumexp_all = singles.tile([P, TILES], F32)
        idx_all = singles.tile([P, TILES, 8], U32)
        tgt2 = singles.tile([P, TILES, 2], I32)
        tgt = singles.tile([P, TILES], I32)
        tlo = singles.tile([P, TILES], I32)
        rowbase = singles.tile([P, TILES], I32)
        goff = singles.tile([P, TILES], I32)
        g = singles.tile([P, TILES], F32)
        pidx = singles.tile([P, TILES], I32)
        woff = singles.tile([P, TILES], I32)
        w = singles.tile([P, TILES], F32)
        lse = singles.tile([P, TILES], F32)
        diff = singles.tile([P, TILES], F32)
        prod = singles.tile([P, TILES], F32)
        partials = singles.tile([P, 2], F32)

        # --- Upfront index plumbing (overlaps the streaming loop) ---------
        # tgt[p, t] = target[128 t + p]  (low int32 word of the int64).
        # Loaded as full pairs (strided-but-contiguous-pair DMA), compacted
        # on-chip so the gather-offset AP is contiguous.
        nc.gpsimd.dma_start(
            out=tgt2[:],
            in_=t32.rearrange("(t p) two -> p t two", p=P),
        )
        # tgt = 2 * low_word(target): element offset of the low int32 word of
        # super_classes[target] in the flat int32 view.  (Multi-element-per-
        # index indirect gathers are broken on HW; single-element gathers are
        # exact, so gather just the low words.)
        nc.vector.tensor_scalar(
            out=tgt[:], in0=tgt2[:, :, 0], scalar1=2, scalar2=None, op0=ALU.mult
        )
        # tlo[p, t] = super_classes[target[...]]  (low int32 word)
        nc.gpsimd.indirect_dma_start(
            out=tlo[:],
            out_offset=None,
            in_=sc32.rearrange("a b -> (a b)")[:, None],
            in_offset=bass.IndirectOffsetOnAxis(ap=tgt[:], axis=0),
        )
        # rowbase[p, t] = (128 t + p) * 1000   (iota steps are int16-limited,
        # so generate the row index first and scale on the vector engine)
        nc.gpsimd.iota(
            rowbase[:], pattern=[[P, TILES]], base=0, channel_multiplier=1
        )
        nc.vector.tensor_scalar(
            out=rowbase[:], in0=rowbase[:], scalar1=C, scalar2=None, op0=ALU.mult
        )
        # goff = rowbase + t_i  -> flat element offset of logits[i, t_i]
        nc.vector.tensor_tensor(
            out=goff[:], in0=tlo[:], in1=rowbase[:], op=ALU.add
        )
        # g[p, t] = logits_flat[goff]
        nc.gpsimd.indirect_dma_start(
            out=g[:],
            out_offset=None,
            in_=x.rearrange("r c -> (r c)")[:, None],
            in_offset=bass.IndirectOffsetOnAxis(ap=goff[:], axis=0),
        )

        # --- Streaming loop over row tiles --------------------------------
        xr = x.rearrange("(t p) c -> t p c", p=P)
        for t in range(TILES):
            xt = xp.tile([P, C], F32)
            nc.sync.dma_start(out=xt[:], in_=xr[t])
            et = ep.tile([P, C], F32)
            nc.scalar.activation(
                out=et[:], in_=xt[:], func=AF.Exp,
                accum_out=sumexp_all[:, t : t + 1],
            )
            v8 = v8p.tile([P, 8], F32)
            nc.vector.max(v8[:], xt[:])
            nc.vector.max_index(idx_all[:, t, :], v8[:], xt[:])

        # --- Tail ----------------------------------------------------------
        # pidx = argmax index as int32
        nc.vector.tensor_copy(out=pidx[:], in_=idx_all[:, :, 0])
        # woff = t_i * 1000 + p_i  (flat offset into class_weights)
        nc.vector.scalar_tensor_tensor(
            out=woff[:], in0=tlo[:], scalar=float(C), in1=pidx[:],
            op0=ALU.mult, op1=ALU.add,
        )
        nc.gpsimd.indirect_dma_start(
            out=w[:],
            out_offset=None,
            in_=cw.rearrange("a b -> (a b)")[:, None],
            in_offset=bass.IndirectOffsetOnAxis(ap=woff[:], axis=0),
        )
        nc.scalar.activation(out=lse[:], in_=sumexp_all[:], func=AF.Ln)
        nc.vector.tensor_tensor(out=diff[:], in0=lse[:], in1=g[:], op=ALU.subtract)
        nc.vector.tensor_tensor(out=prod[:], in0=w[:], in1=diff[:], op=ALU.mult)
        nc.vector.reduce_sum(partials[:, 0:1], prod[:], axis=AX)
        nc.vector.reduce_sum(partials[:, 1:2], w[:], axis=AX)
        nc.sync.dma_start(out=out[:, :], in_=partials[:])


def build_nc(reps=1):
    """reps>1 repeats the whole computation serially (timing calibration)."""
    nc = bacc.Bacc(None, target_bir_lowering=False)
    x = nc.dram_tensor("x", [R, C], F32, kind="ExternalInput")
    t32 = nc.dram_tensor("t32", [R, 2], I32, kind="ExternalInput")
    sc32 = nc.dram_tensor("sc32", [C, 2], I32, kind="ExternalInput")
    cw = nc.dram_tensor("cw", [C, C], F32, kind="ExternalInput")
    out = nc.dram_tensor("partials", [P, 2], F32, kind="ExternalOutput")
    with tile.TileContext(nc) as tc:
        for _ in range(reps):
            build_kernel(tc, x, t32, sc32, cw, out)
    nc.compile()
    return nc


_CACHE = {}


def _get_nc():
    if "nc" not in _CACHE:
        _CACHE["nc"] = build_nc()
    return _CACHE["nc"]


def make_in_maps(logits, class_weights, target, super_classes):
    """Shard the full inputs into per-core input maps (host-side, no math:
    int64 index tensors are byte-reinterpreted as int32 pairs)."""
    logits = np.ascontiguousarray(logits, dtype=np.float32)
    cw = np.ascontiguousarray(class_weights, dtype=np.float32)
    t32 = (
        np.ascontiguousarray(target, dtype=np.int64)
        .view(np.int32)
        .reshape(N, 2)
    )
    sc32 = (
        np.ascontiguousarray(super_classes, dtype=np.int64)
        .view(np.int32)
        .reshape(C, 2)
    )
    in_maps = []
    for c in range(NCORES):
        sl = slice(c * R, (c + 1) * R)
        in_maps.append(
            {
                "x": np.ascontiguousarray(logits[sl]),
                "t32": np.ascontiguousarray(t32[sl]),
                "sc32": sc32,
                "cw": cw,
            }
        )
    return in_maps


def combine(results):
    num = 0.0
    den = 0.0
    for r in results:
        p = r["partials"].astype(np.float64)
        num += p[:, 0].sum()
        den += p[:, 1].sum()
    return np.asarray(np.float32(num / den))


def kernel(logits, class_weights, target, super_classes, _spmd_kwargs=None):
    nc = _get_nc()
    in_maps = make_in_maps(logits, class_weights, target, super_classes)
    kw = dict(_spmd_kwargs or {})
    res = run_bass_kernel_spmd(nc, in_maps, core_ids=list(range(NCORES)), **kw)
    out = combine(res.results)
    if _spmd_kwargs is not None:
        _CACHE["last_results"] = res
    return out

